# revision 36
# baseline (speedup 1.0000x reference)
"""Multi-head causal attention (B=4, S=2048, D=1024, H=16, HD=64) on 8 TRN2 cores.

Strategy:
  - Head-parallel: core i computes heads {2i, 2i+1} for all tokens (bf16
    matmuls throughout; fp8 fails the 2e-2 gate since quantization noise on
    randomly-signed sums passes straight to the output).
  - Startup barrier collective absorbs launch/clock skew before the pipeline;
    the gpsimd queue carries ONLY collectives (a collective blocks its queue
    until completion, so nothing compute-critical may sit behind one).
  - QKV projections grouped q/k/v across both heads (one [128,512] bias add
    per group on DVE).
  - Scores [k,q] with two heads packed via PE row tiling, emitted per
    ki-PAIR with per-parity [128,2,512] PSUM tiles (ring of 2) so the next
    pair's scores overlap this pair's exp. Causal masking is ADDITIVE (-60)
    on PSUM before exp; diagonal parities stream only from their own
    boundary column.
  - exp on ACT with a -2.0 bias; PV accumulates [65,512] per head with a
    ones column in vn (fp-transposed V tiles) producing the softmax
    denominator in row 64.
  - Normalize: den rows -> one K=33 PE matmul broadcast (emat) -> one DVE
    reciprocal -> per-head mult straight from PSUM -> one cc-send per qi.
  - One AllToAll per (batch, half); at-receives prefetched on sync early in
    the next batch; output projection + out stores ride the fill slots.
  - Software pipeline: every ki-pair consumes one "fill" unit (proj groups
    of the next batch, vtrans pairs, oproj units) to keep PE fed during exp
    round trips; b3 runs qi order (2,3,0,1) so the kernel ends on the small
    half; oproj work is deferred two batches so its A2A has always landed.
"""

import sys

sys.path.insert(0, "/opt/trn_rl_repo")

from collections import deque

import numpy as np

import concourse.bass as bass
import concourse.mybir as mybir
import concourse.tile as tile
from concourse import bacc, bass_utils

FP = mybir.dt.float32
BF = mybir.dt.bfloat16
F8 = mybir.dt.float8e4
AOP = mybir.AluOpType
AFT = mybir.ActivationFunctionType
DR = mybir.MatmulPerfMode.DoubleRow

B, S, D, H = 4, 2048, 1024, 16
HD = 64
N_CORES = 8
NT = B * S  # 8192 tokens
TOK_PER_CORE = NT // N_CORES  # 1024
KD = D // 128  # 8 contraction tiles for the projections
VN_BUFS = 10
EXP_BIAS = -2.0
import os
PV_MODE = os.environ.get("PV_MODE", "bf16")  # "bf16" | "fp8" | "dr"
USE_BARRIER = os.environ.get("USE_BARRIER", "1") == "1"
AT_Q = os.environ.get("AT_Q", "sync")  # "gpsimd" | "sync"
DEBUG_DUMP = os.environ.get("DEBUG_DUMP", "0") == "1"
ES_DT = BF if PV_MODE == "bf16" else F8


def build_nc():
    nc = bacc.Bacc(None, target_bir_lowering=False, debug=False, num_devices=N_CORES)

    xt = nc.dram_tensor("xt", [16, 128, KD, 512], BF, kind="ExternalInput")
    wqkv = nc.dram_tensor("wqkv", [128, 3 * KD, 128], BF, kind="ExternalInput")
    bqkv = nc.dram_tensor("bqkv", [128, 3], FP, kind="ExternalInput")
    wo = nc.dram_tensor("wo", [128, KD, D], BF, kind="ExternalInput")
    nmaskd = nc.dram_tensor("nmask", [128, 2, 896], BF, kind="ExternalInput")
    identd = nc.dram_tensor("ident", [128, 128], BF, kind="ExternalInput")
    out = nc.dram_tensor("out", [TOK_PER_CORE, D], FP, kind="ExternalOutput")
    dbg = {}
    if DEBUG_DUMP:
        for nm, shape, dt in [
            ("dbg_q", [128, S], BF), ("dbg_k", [128, S], BF), ("dbg_v", [128, S], BF),
            ("dbg_vn", [128, 2, 2, 72], ES_DT), ("dbg_es", [128, 2, 2, 512], ES_DT),
            ("dbg_dn0", [1, 512], FP), ("dbg_bc", [128, 512], FP),
            ("dbg_rc", [128, 512], FP), ("dbg_on", [128, 512], BF),
            ("dbg_at", [128, N_CORES, 128], BF),
        ]:
            dbg[nm] = nc.dram_tensor(nm, shape, dt, kind="ExternalOutput")

    with tile.TileContext(nc) as tc:
        with (
            tc.tile_pool(name="const", bufs=1) as const,
            tc.tile_pool(name="xtp", bufs=3) as xtp,
            tc.tile_pool(name="qkv", bufs=2) as qkv,
            tc.tile_pool(name="vnp", bufs=VN_BUFS) as vnp,
            tc.tile_pool(name="esp", bufs=2) as esp,
            tc.tile_pool(name="small", bufs=2) as small,
            tc.tile_pool(name="actp", bufs=4) as actp,
            tc.tile_pool(name="oop", bufs=2) as oop,
            tc.tile_pool(name="ps_s", bufs=2, space="PSUM") as ps_s,
            tc.tile_pool(name="ps_o", bufs=1, space="PSUM") as ps_o,
            tc.tile_pool(name="ps_mm", bufs=2, space="PSUM") as ps_mm,
            tc.tile_pool(name="dram", bufs=1, space="DRAM") as dram,
        ):
            cc_ins = [
                [
                    dram.tile([N_CORES, 128, 128], BF, name=f"cc_in{b}_{hf}")
                    for hf in range(2)
                ]
                for b in range(B)
            ]
            cc_outs = [
                [
                    dram.tile([N_CORES, 128, 128], BF, name=f"cc_out{b}_{hf}")
                    for hf in range(2)
                ]
                for b in range(B)
            ]
            bar_i = dram.tile([N_CORES, 128], FP, name="bar_i")
            bar_o = dram.tile([N_CORES, 128], FP, name="bar_o")

            # startup barrier: first collective absorbs launch skew + channel
            # setup while the prologue computes
            if USE_BARRIER:
                nc.gpsimd.collective_compute(
                    "AllToAll",
                    AOP.bypass,
                    replica_groups=[list(range(N_CORES))],
                    ins=[bar_i[:].opt()],
                    outs=[bar_o[:].opt()],
                )

            # PE warmup on junk data while the first DMAs land (HAM ramp)
            warm = const.tile([128, 640], BF, name="warm")
            nc.vector.memset(warm[:], 0.25)
            wps = ps_mm.tile([128, 512], FP, name="wps", tag="mm")
            for w in range(16):
                nc.tensor.matmul(
                    wps[:],
                    lhsT=warm[:, 0:128],
                    rhs=warm[:, 128:640],
                    start=(w == 0),
                    stop=(w == 15),
                )

            # ---- resident constants (scalar queue) ----
            wqkv_sb = const.tile([128, 3 * KD, 128], BF, name="wqkv_sb")
            nc.scalar.dma_start(wqkv_sb[:], wqkv[:])
            bq_sb = const.tile([128, 3], FP, name="bq_sb")
            nc.scalar.dma_start(bq_sb[:], bqkv[:])
            ebias = const.tile([128, 1], FP, name="ebias")
            nc.vector.memset(ebias[:], EXP_BIAS)
            emat = const.tile([33, 128], BF, name="emat")
            nc.vector.memset(emat[:], 0.0)
            nc.vector.memset(emat[0:1, 0:64], 1.0)
            nc.vector.memset(emat[32:33, 64:128], 1.0)
            wo_sb = const.tile([128, KD, D], BF, name="wo_sb")

            qkv_tiles = {}
            vn_tiles = {}
            xt_tiles = {}
            at_tiles = {}

            def alloc_qkv(b):
                qkv_tiles[b] = (
                    qkv.tile([128, S], BF, name="qT", tag="qT"),
                    qkv.tile([128, S], BF, name="kT", tag="kT"),
                    qkv.tile([128, S], BF, name="vT", tag="vT"),
                )

            def emit_xt_load(b, st):
                t = xtp.tile([128, KD, 512], BF, name="xt_st", tag="xt")
                nc.scalar.dma_start(t[:], xt[4 * b + st])
                xt_tiles[(b, st)] = t

            def emit_proj_group(b, st, g):
                xt_st = xt_tiles[(b, st)]
                ps = ps_mm.tile([128, 512], FP, name="ps_p", tag="mm")
                for kd in range(KD):
                    nc.tensor.matmul(
                        ps[:],
                        lhsT=wqkv_sb[:, g * KD + kd, :],
                        rhs=xt_st[:, kd, :],
                        start=(kd == 0),
                        stop=(kd == KD - 1),
                    )
                dst = qkv_tiles[b][g]
                nc.vector.tensor_scalar(
                    dst[:, st * 512 : (st + 1) * 512],
                    ps[:],
                    bq_sb[:, g : g + 1],
                    None,
                    AOP.add,
                )

            def init_vtrans(b):
                vn_tiles[b] = {}

            def emit_vtrans_pair(b, jp):
                _, _, vT = qkv_tiles[b]
                vn = vnp.tile([128, 2, 2, 72], ES_DT, name="vn", tag="vn")
                # fill with 1.0 first; m=64 stays as the denominator ones
                nc.vector.memset(vn[:, :, :, 64:65], 1.0)
                for p in range(2):
                    pst = ps_mm.tile([128, 128], BF, name="ps_t", tag="mm")
                    kc = 2 * jp + p
                    nc.tensor.transpose(
                        pst[:], vT[:, kc * 128 : (kc + 1) * 128], ident_sb[:]
                    )
                    nc.vector.tensor_copy(
                        out=vn[:, p, :, 0:64],
                        in_=pst[:].rearrange("p (h m) -> p h m", h=2),
                    )
                vn_tiles[b][jp] = vn

            def emit_a2a(bb, hf):
                nc.gpsimd.collective_compute(
                    "AllToAll",
                    AOP.bypass,
                    replica_groups=[list(range(N_CORES))],
                    ins=[cc_ins[bb][hf][:].opt()],
                    outs=[cc_outs[bb][hf][:].opt()],
                )


            def emit_at_fetch(bb, pos):
                at = actp.tile([128, N_CORES, 128], BF, name="at", tag="at")
                nc.sync.dma_start(
                    at[:], cc_outs[bb][pos][:].rearrange("f p t -> p f t")
                )
                at_tiles[(bb, pos)] = at

            def emit_oproj_unit(bb, pos, nn):
                at = at_tiles[(bb, pos)]
                if DEBUG_DUMP and bb == 0 and pos == 0 and nn == 0:
                    nc.scalar.dma_start(dbg["dbg_at"][:], at[:])
                ps = ps_mm.tile([128, 512], FP, name="ps_op", tag="mm")
                for ft in range(N_CORES):
                    nc.tensor.matmul(
                        ps[:],
                        lhsT=at[:, ft, :],
                        rhs=wo_sb[:, ft, nn * 512 : (nn + 1) * 512],
                        start=(ft == 0),
                        stop=(ft == N_CORES - 1),
                    )
                row0 = (2 * bb + pos) * 128
                oo = oop.tile([128, 512], FP, name="oo", tag="oo")
                nc.vector.tensor_copy(out=oo[:], in_=ps[:])
                nc.sync.dma_start(
                    out[row0 : row0 + 128, nn * 512 : (nn + 1) * 512], oo[:]
                )

            def emit_pv(b, qi, j, es, cps, po):
                for h in range(2):
                    for p in range(2):
                        nc.tensor.matmul(
                            po[h][:, cps[p] : 512],
                            lhsT=vn_tiles[b][j][:, p, h, 0:65],
                            rhs=es[:, p, h, cps[p] : 512],
                            start=(j == 0 and p == 0),
                            stop=(j == 2 * (qi + 1) - 1 and p == 1),
                        )

            def emit_attn_qi(b, qi, fill):
                qT, kT, _ = qkv_tiles[b]
                po = [
                    ps_o.tile([65, 512], FP, name=f"po{h}", tag=f"o{h}")
                    for h in range(2)
                ]
                NP = 2 * (qi + 1)
                prev = None
                for j in range(NP):
                    jr = j - 2 * qi
                    # per-parity first column: even starts at its boundary,
                    # odd one 128-block later
                    if jr >= 0:
                        cps = (256 * jr, 256 * jr + 128)
                    else:
                        cps = (0, 0)
                    es = esp.tile([128, 2, 2, 512], ES_DT, name="es", tag="es")
                    for p in range(2):
                        # per-parity psum tile; bufs=2 ring lets scores of the
                        # next pair overlap this pair's exp
                        sc = ps_s.tile([128, 2, 512], FP, name="sc", tag="sc")
                        ki = 2 * j + p
                        for h in range(2):
                            nc.tensor.matmul(
                                sc[:, h, cps[p] : 512],
                                lhsT=kT[h * 64 : (h + 1) * 64, ki * 128 : (ki + 1) * 128],
                                rhs=qT[
                                    h * 64 : (h + 1) * 64,
                                    qi * 512 + cps[p] : (qi + 1) * 512,
                                ],
                                start=True,
                                stop=True,
                                tile_position=(h * 64, 0),
                            )
                        if jr >= 0:
                            nc.vector.tensor_tensor(
                                sc[:, :, cps[p] : cps[p] + 128],
                                sc[:, :, cps[p] : cps[p] + 128],
                                nmask_sb[:, :, 384:512],
                                AOP.add,
                            )
                        nc.scalar.activation(
                            es[:, p, :, cps[p] : 512],
                            sc[:, :, cps[p] : 512],
                            AFT.Exp,
                            bias=ebias[:],
                        )
                    if DEBUG_DUMP and b == 0 and qi == 0 and j == 0:
                        nc.scalar.dma_start(dbg["dbg_es"][:], es[:])
                    if prev is not None:
                        emit_pv(b, qi, j - 1, prev[0], prev[1], po)
                    prev = (es, cps)
                    if fill:
                        u = fill.popleft()
                        if u is not None:
                            u()
                emit_pv(b, qi, NP - 1, prev[0], prev[1], po)

                # normalize: den -> broadcast -> reciprocal -> scale
                dnb = small.tile([33, 512], BF, name="dnb", tag="dnb")
                nc.vector.memset(dnb[:], 0.0)
                nc.vector.tensor_scalar_max(dnb[0:1, :], po[0][64:65, :], 1e-30)
                nc.vector.tensor_scalar_max(dnb[32:33, :], po[1][64:65, :], 1e-30)
                # one matmul broadcasts den0 -> rows 0-63, den1 -> rows 64-127
                bcb = ps_mm.tile([128, 512], FP, name="bcb", tag="mm")
                nc.tensor.matmul(bcb[:], lhsT=emat[:], rhs=dnb[:], start=True, stop=True)
                rc = small.tile([128, 512], FP, name="rc", tag="rc")
                nc.vector.reciprocal_approx_fast(out=rc[:], in_=bcb[:])
                on = small.tile([128, 512], BF, name="on", tag="on")
                nc.vector.tensor_tensor(on[0:64, :], po[0][0:64, :], rc[0:64, :], AOP.mult)
                nc.vector.tensor_tensor(
                    on[64:128, :], po[1][0:64, :], rc[64:128, :], AOP.mult
                )
                if DEBUG_DUMP and b == 0 and qi == 0:
                    nc.scalar.dma_start(dbg["dbg_dn0"][:], dnb[0:1, :])
                    nc.scalar.dma_start(dbg["dbg_rc"][:], rc[:])
                    nc.scalar.dma_start(dbg["dbg_on"][:], on[:])
                t0r = (4 * qi) % 8
                nc.sync.dma_start(
                    cc_ins[b][qi // 2][t0r : t0r + 4, :, :].rearrange("r p t -> p r t"),
                    on[:].rearrange("p (r t) -> p r t", r=4),
                )

            # ---- prologue: only st0 of batch 0; the rest rides b0's fill
            alloc_qkv(0)
            init_vtrans(0)
            emit_xt_load(0, 0)
            nmask_sb = const.tile([128, 2, 896], BF, name="nmask_sb")
            nc.scalar.dma_start(nmask_sb[:], nmaskd[:])
            ident_sb = const.tile([128, 128], BF, name="ident_sb")
            nc.scalar.dma_start(ident_sb[:], identd[:])
            emit_xt_load(0, 1)
            for g in range(3):
                emit_proj_group(0, 0, g)
            nc.scalar.dma_start(wo_sb[:], wo[:])
            for jp in range(2):
                emit_vtrans_pair(0, jp)
            if DEBUG_DUMP:
                nc.scalar.dma_start(dbg["dbg_q"][:], qkv_tiles[0][0][:])
                nc.scalar.dma_start(dbg["dbg_k"][:], qkv_tiles[0][1][:])
                nc.scalar.dma_start(dbg["dbg_v"][:], qkv_tiles[0][2][:])
                nc.scalar.dma_start(dbg["dbg_vn"][:], vn_tiles[0][0][:])

            # ---- software-pipelined batch loop ----
            # fill slots per batch: sum NP = 2+4+6+8 = 20, consumed one per
            # ki-pair; keep PE fed during the exp round trips
            for b in range(B):
                last = b == B - 1
                fill = deque()
                if b == 0:
                    for st in (1, 2, 3):
                        def u_load0(st=st):
                            if st < 3:
                                emit_xt_load(0, st + 1)
                            emit_proj_group(0, st, 0)
                        fill.append(u_load0)
                        for g in (1, 2):
                            fill.append(lambda st=st, g=g: emit_proj_group(0, st, g))
                        fill.append(lambda st=st: emit_vtrans_pair(0, 2 * st))
                        fill.append(lambda st=st: emit_vtrans_pair(0, 2 * st + 1))
                if not last:
                    alloc_qkv(b + 1)
                    init_vtrans(b + 1)
                    for st in range(4):
                        def u_load(st=st):
                            emit_xt_load(b + 1, st)
                            emit_proj_group(b + 1, st, 0)
                        fill.append(u_load)
                        for g in (1, 2):
                            fill.append(lambda st=st, g=g: emit_proj_group(b + 1, st, g))
                    # oproj for (b-2,1) and (b-1,0): both A2As long done
                    if b >= 2:
                        fill.appendleft(lambda: emit_at_fetch(b - 2, 1))
                        for nn in range(2):
                            fill.append(lambda nn=nn: emit_oproj_unit(b - 2, 1, nn))
                    if b >= 1:
                        fill.appendleft(lambda: emit_at_fetch(b - 1, 0))
                        for nn in range(2):
                            fill.append(lambda nn=nn: emit_oproj_unit(b - 1, 0, nn))
                else:
                    # b3 (qi order 2,3,0,1): all pending oproj, oldest first,
                    # spaced so each unit's A2A has landed before its slot
                    fill.append(lambda: emit_at_fetch(1, 1))
                    for nn in range(2):
                        fill.append(lambda nn=nn: emit_oproj_unit(1, 1, nn))
                    fill.append(lambda: emit_at_fetch(2, 0))
                    for nn in range(2):
                        fill.append(lambda nn=nn: emit_oproj_unit(2, 0, nn))
                    fill.extend([None] * 2)
                    fill.append(lambda: emit_at_fetch(2, 1))
                    for nn in range(2):
                        fill.append(lambda nn=nn: emit_oproj_unit(2, 1, nn))
                    fill.extend([None] * 6)
                    fill.append(lambda: emit_at_fetch(3, 1))
                    for nn in range(2):
                        fill.append(lambda nn=nn: emit_oproj_unit(3, 1, nn))
                qi_order = (2, 3, 0, 1) if last else (0, 1, 2, 3)
                for qi in qi_order:
                    emit_attn_qi(b, qi, fill)
                    if qi == 1:
                        emit_a2a(b, 0)
                    if last and qi == 3:
                        emit_a2a(b, 1)
                if not last:
                    emit_a2a(b, 1)
                while fill:
                    u = fill.popleft()
                    if u is not None:
                        u()
                if not last:
                    for jp in range(8):
                        emit_vtrans_pair(b + 1, jp)
            emit_at_fetch(B - 1, 0)
            for nn in range(2):
                emit_oproj_unit(B - 1, 0, nn)

    nc.finalize()
    return nc


_NC_CACHE = None


def _get_nc():
    global _NC_CACHE
    if _NC_CACHE is None:
        _NC_CACHE = build_nc()
    return _NC_CACHE


def make_in_maps(x, Wqkv, bqkv, Wo):
    import ml_dtypes

    bf16 = ml_dtypes.bfloat16
    scale = HD ** -0.5
    xT = x.reshape(NT, D).T.astype(bf16)  # [D, NT]
    xtn = np.ascontiguousarray(
        xT.reshape(KD, 128, 16, 512).transpose(2, 1, 0, 3)
    )  # [slab, p, kd, t]
    maskb = np.arange(896)[None, :] - 384 >= np.arange(128)[:, None]
    nmask = np.where(maskb, 0.0, -60.0).astype(bf16)
    nmask2 = np.ascontiguousarray(np.stack([nmask, nmask], axis=1))  # [128,2,896]
    ident = np.eye(128, dtype=np.float32).astype(bf16)
    wo = np.ascontiguousarray(Wo.astype(bf16).reshape(KD, 128, D).transpose(1, 0, 2))
    in_maps = []
    for c in range(N_CORES):
        h0, h1 = 2 * c, 2 * c + 1
        # grouped q/k/v weights across both heads: [3, D, 128]
        wq = np.concatenate([Wqkv[h0][:, 0:64] * scale, Wqkv[h1][:, 0:64] * scale], 1)
        wk = np.concatenate([Wqkv[h0][:, 64:128], Wqkv[h1][:, 64:128]], 1)
        wv = np.concatenate([Wqkv[h0][:, 128:192], Wqkv[h1][:, 128:192]], 1)
        wg = np.stack([wq, wk, wv]).astype(bf16)  # [3, D, 128]
        wg = (
            wg.reshape(3, KD, 128, 128).transpose(2, 0, 1, 3).reshape(128, 3 * KD, 128)
        )
        bq = np.concatenate([bqkv[h0][0:64] * scale, bqkv[h1][0:64] * scale])
        bk = np.concatenate([bqkv[h0][64:128], bqkv[h1][64:128]])
        bv = np.concatenate([bqkv[h0][128:192], bqkv[h1][128:192]])
        bg = np.stack([bq, bk, bv], axis=1).astype(np.float32)  # [128, 3]
        in_maps.append(
            {
                "xt": xtn,
                "wqkv": np.ascontiguousarray(wg),
                "bqkv": np.ascontiguousarray(bg),
                "wo": wo,
                "nmask": nmask2,
                "ident": ident,
            }
        )
    return in_maps


def run_cores(in_maps, trace=False, trace_kwargs=None):
    nc = _get_nc()
    kwargs = {}
    if trace:
        kwargs["trace"] = True
        if trace_kwargs:
            kwargs["trace_kwargs"] = trace_kwargs
    return bass_utils.run_bass_kernel_spmd(
        nc, in_maps, core_ids=list(range(N_CORES)), **kwargs
    )


def assemble(results, bo):
    """Reassemble core outputs (interleaved token-tile mapping) into [B,S,D]."""
    full = np.empty((NT, D), np.float32)
    for c in range(N_CORES):
        o = results[c]["out"]
        for b in range(B):
            for pos in range(2):
                t = c + 8 * pos  # token tile within batch
                dst = b * S + t * 128
                full[dst : dst + 128] = o[(2 * b + pos) * 128 : (2 * b + pos + 1) * 128]
    full += bo[None, :]
    return full.reshape(B, S, D)


def kernel(x, Wqkv, bqkv, Wo, bo):
    x = np.asarray(x, dtype=np.float32)
    Wqkv = np.asarray(Wqkv, dtype=np.float32)
    bqkv = np.asarray(bqkv, dtype=np.float32)
    Wo = np.asarray(Wo, dtype=np.float32)
    bo = np.asarray(bo, dtype=np.float32)

    in_maps = make_in_maps(x, Wqkv, bqkv, Wo)
    res = run_cores(in_maps)
    return assemble(res.results, bo)


# revision 38
# speedup vs baseline: 1.0227x; 1.0227x over previous
"""Multi-head causal attention (B=4, S=2048, D=1024, H=16, HD=64) on 8 TRN2 cores.

Strategy:
  - Head-parallel: core i computes heads {2i, 2i+1} for all tokens (bf16
    matmuls throughout; fp8 fails the 2e-2 gate since quantization noise on
    randomly-signed sums passes straight to the output).
  - Startup barrier collective absorbs launch/clock skew before the pipeline;
    the gpsimd queue carries ONLY collectives (a collective blocks its queue
    until completion, so nothing compute-critical may sit behind one).
  - QKV projections grouped q/k/v across both heads (one [128,512] bias add
    per group on DVE).
  - Scores [k,q] with two heads packed via PE row tiling, emitted per
    ki-PAIR with per-parity [128,2,512] PSUM tiles (ring of 2) so the next
    pair's scores overlap this pair's exp. Causal masking is ADDITIVE (-60)
    on PSUM before exp; diagonal parities stream only from their own
    boundary column.
  - exp on ACT with a -2.0 bias; PV accumulates [65,512] per head with a
    ones column in vn (fp-transposed V tiles) producing the softmax
    denominator in row 64.
  - Normalize: den rows -> one K=33 PE matmul broadcast (emat) -> one DVE
    reciprocal -> per-head mult straight from PSUM -> one cc-send per qi.
  - One AllToAll per (batch, half); at-receives prefetched on sync early in
    the next batch; output projection + out stores ride the fill slots.
  - Software pipeline: every ki-pair consumes one "fill" unit (proj groups
    of the next batch, vtrans pairs, oproj units) to keep PE fed during exp
    round trips; b3 runs qi order (2,3,0,1) so the kernel ends on the small
    half; oproj work is deferred two batches so its A2A has always landed.
"""

import sys

sys.path.insert(0, "/opt/trn_rl_repo")

from collections import deque

import numpy as np

import concourse.bass as bass
import concourse.mybir as mybir
import concourse.tile as tile
from concourse import bacc, bass_utils

FP = mybir.dt.float32
BF = mybir.dt.bfloat16
F8 = mybir.dt.float8e4
AOP = mybir.AluOpType
AFT = mybir.ActivationFunctionType
DR = mybir.MatmulPerfMode.DoubleRow

B, S, D, H = 4, 2048, 1024, 16
HD = 64
N_CORES = 8
NT = B * S  # 8192 tokens
TOK_PER_CORE = NT // N_CORES  # 1024
KD = D // 128  # 8 contraction tiles for the projections
VN_BUFS = 10
EXP_BIAS = -2.0
import os
PV_MODE = os.environ.get("PV_MODE", "bf16")  # "bf16" | "fp8" | "dr"
USE_BARRIER = os.environ.get("USE_BARRIER", "1") == "1"
AT_Q = os.environ.get("AT_Q", "sync")  # "gpsimd" | "sync"
DEBUG_DUMP = os.environ.get("DEBUG_DUMP", "0") == "1"
ES_DT = BF if PV_MODE == "bf16" else F8


def build_nc():
    nc = bacc.Bacc(None, target_bir_lowering=False, debug=False, num_devices=N_CORES)

    xt = nc.dram_tensor("xt", [16, 128, KD, 512], BF, kind="ExternalInput")
    wqkv = nc.dram_tensor("wqkv", [128, 3 * KD, 128], BF, kind="ExternalInput")
    bqkv = nc.dram_tensor("bqkv", [128, 3], FP, kind="ExternalInput")
    wo = nc.dram_tensor("wo", [128, KD, D], BF, kind="ExternalInput")
    nmaskd = nc.dram_tensor("nmask", [128, 2, 896], BF, kind="ExternalInput")
    identd = nc.dram_tensor("ident", [128, 128], BF, kind="ExternalInput")
    out = nc.dram_tensor("out", [TOK_PER_CORE, D], FP, kind="ExternalOutput")
    dbg = {}
    if DEBUG_DUMP:
        for nm, shape, dt in [
            ("dbg_q", [128, S], BF), ("dbg_k", [128, S], BF), ("dbg_v", [128, S], BF),
            ("dbg_vn", [128, 2, 2, 72], ES_DT), ("dbg_es", [128, 2, 2, 512], ES_DT),
            ("dbg_dn0", [1, 512], FP), ("dbg_bc", [128, 512], FP),
            ("dbg_rc", [128, 512], FP), ("dbg_on", [128, 512], BF),
            ("dbg_at", [128, N_CORES, 128], BF),
        ]:
            dbg[nm] = nc.dram_tensor(nm, shape, dt, kind="ExternalOutput")

    with tile.TileContext(nc) as tc:
        with (
            tc.tile_pool(name="const", bufs=1) as const,
            tc.tile_pool(name="xtp", bufs=3) as xtp,
            tc.tile_pool(name="qkv", bufs=2) as qkv,
            tc.tile_pool(name="vnp", bufs=VN_BUFS) as vnp,
            tc.tile_pool(name="esp", bufs=2) as esp,
            tc.tile_pool(name="small", bufs=2) as small,
            tc.tile_pool(name="actp", bufs=4) as actp,
            tc.tile_pool(name="oop", bufs=2) as oop,
            tc.tile_pool(name="ps_s", bufs=2, space="PSUM") as ps_s,
            tc.tile_pool(name="ps_o", bufs=1, space="PSUM") as ps_o,
            tc.tile_pool(name="ps_mm", bufs=2, space="PSUM") as ps_mm,
            tc.tile_pool(name="dram", bufs=1, space="DRAM") as dram,
        ):
            cc_ins = [
                [
                    dram.tile([N_CORES, 128, 128], BF, name=f"cc_in{b}_{hf}")
                    for hf in range(2)
                ]
                for b in range(B)
            ]
            cc_outs = [
                [
                    dram.tile([N_CORES, 128, 128], BF, name=f"cc_out{b}_{hf}")
                    for hf in range(2)
                ]
                for b in range(B)
            ]
            bar_i = dram.tile([N_CORES, 128], FP, name="bar_i")
            bar_o = dram.tile([N_CORES, 128], FP, name="bar_o")

            # startup barrier: first collective absorbs launch skew + channel
            # setup while the prologue computes
            if USE_BARRIER:
                nc.gpsimd.collective_compute(
                    "AllToAll",
                    AOP.bypass,
                    replica_groups=[list(range(N_CORES))],
                    ins=[bar_i[:].opt()],
                    outs=[bar_o[:].opt()],
                )

            # PE warmup on junk data while the first DMAs land (HAM ramp)
            warm = const.tile([128, 640], BF, name="warm")
            nc.vector.memset(warm[:], 0.25)
            wps = ps_mm.tile([128, 512], FP, name="wps", tag="mm")
            for w in range(16):
                nc.tensor.matmul(
                    wps[:],
                    lhsT=warm[:, 0:128],
                    rhs=warm[:, 128:640],
                    start=(w == 0),
                    stop=(w == 15),
                )

            # ---- resident constants (scalar queue) ----
            wqkv_sb = const.tile([128, 3 * KD, 128], BF, name="wqkv_sb")
            nc.scalar.dma_start(wqkv_sb[:], wqkv[:])
            bq_sb = const.tile([128, 3], FP, name="bq_sb")
            nc.scalar.dma_start(bq_sb[:], bqkv[:])
            ebias = const.tile([128, 1], FP, name="ebias")
            nc.vector.memset(ebias[:], EXP_BIAS)
            emat = const.tile([33, 128], BF, name="emat")
            nc.vector.memset(emat[:], 0.0)
            nc.vector.memset(emat[0:1, 0:64], 1.0)
            nc.vector.memset(emat[32:33, 64:128], 1.0)
            wo_sb = const.tile([128, KD, D], BF, name="wo_sb")

            qkv_tiles = {}
            vn_tiles = {}
            xt_tiles = {}
            at_tiles = {}

            def alloc_qkv(b):
                qkv_tiles[b] = (
                    qkv.tile([128, S], BF, name="qT", tag="qT"),
                    qkv.tile([128, S], BF, name="kT", tag="kT"),
                    qkv.tile([128, S], BF, name="vT", tag="vT"),
                )

            def emit_xt_load(b, st):
                t = xtp.tile([128, KD, 512], BF, name="xt_st", tag="xt")
                nc.scalar.dma_start(t[:], xt[4 * b + st])
                xt_tiles[(b, st)] = t

            def emit_proj_group(b, st, g):
                xt_st = xt_tiles[(b, st)]
                ps = ps_mm.tile([128, 512], FP, name="ps_p", tag="mm")
                for kd in range(KD):
                    nc.tensor.matmul(
                        ps[:],
                        lhsT=wqkv_sb[:, g * KD + kd, :],
                        rhs=xt_st[:, kd, :],
                        start=(kd == 0),
                        stop=(kd == KD - 1),
                    )
                dst = qkv_tiles[b][g]
                nc.vector.tensor_scalar(
                    dst[:, st * 512 : (st + 1) * 512],
                    ps[:],
                    bq_sb[:, g : g + 1],
                    None,
                    AOP.add,
                )

            def init_vtrans(b):
                vn_tiles[b] = {}

            def emit_vtrans_pair(b, jp):
                _, _, vT = qkv_tiles[b]
                vn = vnp.tile([128, 2, 2, 72], ES_DT, name="vn", tag="vn")
                # fill with 1.0 first; m=64 stays as the denominator ones
                nc.vector.memset(vn[:, :, :, 64:65], 1.0)
                for p in range(2):
                    pst = ps_mm.tile([128, 128], BF, name="ps_t", tag="mm")
                    kc = 2 * jp + p
                    nc.tensor.transpose(
                        pst[:], vT[:, kc * 128 : (kc + 1) * 128], ident_sb[:]
                    )
                    nc.vector.tensor_copy(
                        out=vn[:, p, :, 0:64],
                        in_=pst[:].rearrange("p (h m) -> p h m", h=2),
                    )
                vn_tiles[b][jp] = vn

            def emit_a2a(bb, hf):
                nc.gpsimd.collective_compute(
                    "AllToAll",
                    AOP.bypass,
                    replica_groups=[list(range(N_CORES))],
                    ins=[cc_ins[bb][hf][:].opt()],
                    outs=[cc_outs[bb][hf][:].opt()],
                )


            def emit_at_fetch(bb, pos):
                at = actp.tile([128, N_CORES, 128], BF, name="at", tag="at")
                nc.sync.dma_start(
                    at[:], cc_outs[bb][pos][:].rearrange("f p t -> p f t")
                )
                at_tiles[(bb, pos)] = at

            def emit_oproj_unit(bb, pos, nn):
                at = at_tiles[(bb, pos)]
                if DEBUG_DUMP and bb == 0 and pos == 0 and nn == 0:
                    nc.scalar.dma_start(dbg["dbg_at"][:], at[:])
                ps = ps_mm.tile([128, 512], FP, name="ps_op", tag="mm")
                for ft in range(N_CORES):
                    nc.tensor.matmul(
                        ps[:],
                        lhsT=at[:, ft, :],
                        rhs=wo_sb[:, ft, nn * 512 : (nn + 1) * 512],
                        start=(ft == 0),
                        stop=(ft == N_CORES - 1),
                    )
                row0 = (2 * bb + pos) * 128
                oo = oop.tile([128, 512], FP, name="oo", tag="oo")
                nc.vector.tensor_copy(out=oo[:], in_=ps[:])
                nc.sync.dma_start(
                    out[row0 : row0 + 128, nn * 512 : (nn + 1) * 512], oo[:]
                )

            def emit_pv(b, qi, j, es, cps, po):
                for h in range(2):
                    for p in range(2):
                        nc.tensor.matmul(
                            po[h][:, cps[p] : 512],
                            lhsT=vn_tiles[b][j][:, p, h, 0:65],
                            rhs=es[:, p, h, cps[p] : 512],
                            start=(j == 0 and p == 0),
                            stop=(j == 2 * (qi + 1) - 1 and p == 1),
                        )

            def emit_attn_qi(b, qi, fill):
                qT, kT, _ = qkv_tiles[b]
                po = [
                    ps_o.tile([65, 512], FP, name=f"po{h}", tag=f"o{h}")
                    for h in range(2)
                ]
                NP = 2 * (qi + 1)
                prev = None
                for j in range(NP):
                    jr = j - 2 * qi
                    # per-parity first column: even starts at its boundary,
                    # odd one 128-block later
                    if jr >= 0:
                        cps = (256 * jr, 256 * jr + 128)
                    else:
                        cps = (0, 0)
                    es = esp.tile([128, 2, 2, 512], ES_DT, name="es", tag="es")
                    for p in range(2):
                        # per-parity psum tile; bufs=2 ring lets scores of the
                        # next pair overlap this pair's exp
                        sc = ps_s.tile([128, 2, 512], FP, name="sc", tag="sc")
                        ki = 2 * j + p
                        for h in range(2):
                            nc.tensor.matmul(
                                sc[:, h, cps[p] : 512],
                                lhsT=kT[h * 64 : (h + 1) * 64, ki * 128 : (ki + 1) * 128],
                                rhs=qT[
                                    h * 64 : (h + 1) * 64,
                                    qi * 512 + cps[p] : (qi + 1) * 512,
                                ],
                                start=True,
                                stop=True,
                                tile_position=(h * 64, 0),
                            )
                        if jr >= 0:
                            nc.vector.tensor_tensor(
                                sc[:, :, cps[p] : cps[p] + 128],
                                sc[:, :, cps[p] : cps[p] + 128],
                                nmask_sb[:, :, 384:512],
                                AOP.add,
                            )
                        nc.scalar.activation(
                            es[:, p, :, cps[p] : 512],
                            sc[:, :, cps[p] : 512],
                            AFT.Exp,
                            bias=ebias[:],
                        )
                    if DEBUG_DUMP and b == 0 and qi == 0 and j == 0:
                        nc.scalar.dma_start(dbg["dbg_es"][:], es[:])
                    if prev is not None:
                        emit_pv(b, qi, j - 1, prev[0], prev[1], po)
                    prev = (es, cps)
                    if fill:
                        u = fill.popleft()
                        if u is not None:
                            u()
                emit_pv(b, qi, NP - 1, prev[0], prev[1], po)

                # normalize: den -> broadcast -> reciprocal -> scale
                dnb = small.tile([33, 512], BF, name="dnb", tag="dnb")
                nc.vector.memset(dnb[:], 0.0)
                nc.vector.tensor_scalar_max(dnb[0:1, :], po[0][64:65, :], 1e-30)
                nc.vector.tensor_scalar_max(dnb[32:33, :], po[1][64:65, :], 1e-30)
                # one matmul broadcasts den0 -> rows 0-63, den1 -> rows 64-127
                bcb = ps_mm.tile([128, 512], FP, name="bcb", tag="mm")
                nc.tensor.matmul(bcb[:], lhsT=emat[:], rhs=dnb[:], start=True, stop=True)
                rc = small.tile([128, 512], FP, name="rc", tag="rc")
                nc.vector.reciprocal_approx_fast(out=rc[:], in_=bcb[:])
                on = small.tile([128, 512], BF, name="on", tag="on")
                nc.vector.tensor_tensor(on[0:64, :], po[0][0:64, :], rc[0:64, :], AOP.mult)
                nc.vector.tensor_tensor(
                    on[64:128, :], po[1][0:64, :], rc[64:128, :], AOP.mult
                )
                if DEBUG_DUMP and b == 0 and qi == 0:
                    nc.scalar.dma_start(dbg["dbg_dn0"][:], dnb[0:1, :])
                    nc.scalar.dma_start(dbg["dbg_rc"][:], rc[:])
                    nc.scalar.dma_start(dbg["dbg_on"][:], on[:])
                t0r = (4 * qi) % 8
                nc.sync.dma_start(
                    cc_ins[b][qi // 2][t0r : t0r + 4, :, :].rearrange("r p t -> p r t"),
                    on[:].rearrange("p (r t) -> p r t", r=4),
                )

            # ---- prologue: only st0 of batch 0; the rest rides b0's fill
            alloc_qkv(0)
            init_vtrans(0)
            emit_xt_load(0, 0)
            nmask_sb = const.tile([128, 2, 896], BF, name="nmask_sb")
            nc.scalar.dma_start(nmask_sb[:], nmaskd[:])
            ident_sb = const.tile([128, 128], BF, name="ident_sb")
            nc.scalar.dma_start(ident_sb[:], identd[:])
            emit_xt_load(0, 1)
            for g in range(3):
                emit_proj_group(0, 0, g)
            nc.scalar.dma_start(wo_sb[:], wo[:])
            for jp in range(2):
                emit_vtrans_pair(0, jp)
            if DEBUG_DUMP:
                nc.scalar.dma_start(dbg["dbg_q"][:], qkv_tiles[0][0][:])
                nc.scalar.dma_start(dbg["dbg_k"][:], qkv_tiles[0][1][:])
                nc.scalar.dma_start(dbg["dbg_v"][:], qkv_tiles[0][2][:])
                nc.scalar.dma_start(dbg["dbg_vn"][:], vn_tiles[0][0][:])

            # ---- software-pipelined batch loop ----
            # fill slots per batch: sum NP = 2+4+6+8 = 20, consumed one per
            # ki-pair; keep PE fed during the exp round trips
            for b in range(B):
                last = b == B - 1
                fill = deque()
                if b == 0:
                    for st in (1, 2, 3):
                        def u_load0(st=st):
                            if st < 3:
                                emit_xt_load(0, st + 1)
                            emit_proj_group(0, st, 0)
                        fill.append(u_load0)
                        for g in (1, 2):
                            fill.append(lambda st=st, g=g: emit_proj_group(0, st, g))
                        fill.append(lambda st=st: emit_vtrans_pair(0, 2 * st))
                        fill.append(lambda st=st: emit_vtrans_pair(0, 2 * st + 1))
                if not last:
                    alloc_qkv(b + 1)
                    init_vtrans(b + 1)
                    for st in range(4):
                        def u_load(st=st):
                            emit_xt_load(b + 1, st)
                            emit_proj_group(b + 1, st, 0)
                        fill.append(u_load)
                        for g in (1, 2):
                            fill.append(lambda st=st, g=g: emit_proj_group(b + 1, st, g))
                    # oproj for (b-2,1) and (b-1,0): both A2As long done
                    if b >= 2:
                        fill.appendleft(lambda: emit_at_fetch(b - 2, 1))
                        for nn in range(2):
                            fill.append(lambda nn=nn: emit_oproj_unit(b - 2, 1, nn))
                    if b >= 1:
                        fill.appendleft(lambda: emit_at_fetch(b - 1, 0))
                        for nn in range(2):
                            fill.append(lambda nn=nn: emit_oproj_unit(b - 1, 0, nn))
                else:
                    # b3 (qi order 2,3,0,1): all pending oproj, oldest first,
                    # spaced so each unit's A2A has landed before its slot
                    fill.append(lambda: emit_at_fetch(1, 1))
                    for nn in range(2):
                        fill.append(lambda nn=nn: emit_oproj_unit(1, 1, nn))
                    fill.append(lambda: emit_at_fetch(2, 0))
                    for nn in range(2):
                        fill.append(lambda nn=nn: emit_oproj_unit(2, 0, nn))
                    fill.extend([None] * 2)
                    fill.append(lambda: emit_at_fetch(2, 1))
                    for nn in range(2):
                        fill.append(lambda nn=nn: emit_oproj_unit(2, 1, nn))
                    fill.extend([None] * 6)
                    fill.append(lambda: emit_at_fetch(3, 1))
                    for nn in range(2):
                        fill.append(lambda nn=nn: emit_oproj_unit(3, 1, nn))
                qi_order = (2, 3, 0, 1) if last else (0, 1, 2, 3)
                for qi in qi_order:
                    emit_attn_qi(b, qi, fill)
                    if qi == 1:
                        emit_a2a(b, 0)
                    if last and qi == 3:
                        emit_a2a(b, 1)
                if not last:
                    emit_a2a(b, 1)
                while fill:
                    u = fill.popleft()
                    if u is not None:
                        u()
                if not last:
                    for jp in range(8):
                        emit_vtrans_pair(b + 1, jp)
            emit_at_fetch(B - 1, 0)
            for nn in range(2):
                emit_oproj_unit(B - 1, 0, nn)

    nc.finalize()
    return nc


_NC_CACHE = None


def _get_nc():
    global _NC_CACHE
    if _NC_CACHE is None:
        _NC_CACHE = build_nc()
    return _NC_CACHE


def make_in_maps(x, Wqkv, bqkv, Wo):
    import ml_dtypes

    bf16 = ml_dtypes.bfloat16
    scale = HD ** -0.5
    xT = x.reshape(NT, D).T.astype(bf16)  # [D, NT]
    xtn = np.ascontiguousarray(
        xT.reshape(KD, 128, 16, 512).transpose(2, 1, 0, 3)
    )  # [slab, p, kd, t]
    maskb = np.arange(896)[None, :] - 384 >= np.arange(128)[:, None]
    nmask = np.where(maskb, 0.0, -60.0).astype(bf16)
    nmask2 = np.ascontiguousarray(np.stack([nmask, nmask], axis=1))  # [128,2,896]
    ident = np.eye(128, dtype=np.float32).astype(bf16)
    wo = np.ascontiguousarray(Wo.astype(bf16).reshape(KD, 128, D).transpose(1, 0, 2))
    in_maps = []
    for c in range(N_CORES):
        h0, h1 = 2 * c, 2 * c + 1
        # grouped q/k/v weights across both heads: [3, D, 128]
        wq = np.concatenate([Wqkv[h0][:, 0:64] * scale, Wqkv[h1][:, 0:64] * scale], 1)
        wk = np.concatenate([Wqkv[h0][:, 64:128], Wqkv[h1][:, 64:128]], 1)
        wv = np.concatenate([Wqkv[h0][:, 128:192], Wqkv[h1][:, 128:192]], 1)
        wg = np.stack([wq, wk, wv]).astype(bf16)  # [3, D, 128]
        wg = (
            wg.reshape(3, KD, 128, 128).transpose(2, 0, 1, 3).reshape(128, 3 * KD, 128)
        )
        bq = np.concatenate([bqkv[h0][0:64] * scale, bqkv[h1][0:64] * scale])
        bk = np.concatenate([bqkv[h0][64:128], bqkv[h1][64:128]])
        bv = np.concatenate([bqkv[h0][128:192], bqkv[h1][128:192]])
        bg = np.stack([bq, bk, bv], axis=1).astype(np.float32)  # [128, 3]
        in_maps.append(
            {
                "xt": xtn,
                "wqkv": np.ascontiguousarray(wg),
                "bqkv": np.ascontiguousarray(bg),
                "wo": wo,
                "nmask": nmask2,
                "ident": ident,
            }
        )
    return in_maps


def run_cores(in_maps, trace=False, trace_kwargs=None):
    nc = _get_nc()
    kwargs = {}
    if trace:
        kwargs["trace"] = True
        if trace_kwargs:
            kwargs["trace_kwargs"] = trace_kwargs
    return bass_utils.run_bass_kernel_spmd(
        nc, in_maps, core_ids=list(range(N_CORES)), **kwargs
    )


def assemble(results, bo):
    """Reassemble core outputs (interleaved token-tile mapping) into [B,S,D]."""
    full = np.empty((NT, D), np.float32)
    for c in range(N_CORES):
        o = results[c]["out"]
        for b in range(B):
            for pos in range(2):
                t = c + 8 * pos  # token tile within batch
                dst = b * S + t * 128
                full[dst : dst + 128] = o[(2 * b + pos) * 128 : (2 * b + pos + 1) * 128]
    full += bo[None, :]
    return full.reshape(B, S, D)


def kernel(x, Wqkv, bqkv, Wo, bo):
    x = np.asarray(x, dtype=np.float32)
    Wqkv = np.asarray(Wqkv, dtype=np.float32)
    bqkv = np.asarray(bqkv, dtype=np.float32)
    Wo = np.asarray(Wo, dtype=np.float32)
    bo = np.asarray(bo, dtype=np.float32)

    in_maps = make_in_maps(x, Wqkv, bqkv, Wo)
    res = run_cores(in_maps)
    return assemble(res.results, bo)


# revision 39
# speedup vs baseline: 1.0310x; 1.0081x over previous
"""Multi-head causal attention (B=4, S=2048, D=1024, H=16, HD=64) on 8 TRN2 cores.

Strategy:
  - Head-parallel: core i computes heads {2i, 2i+1} for all tokens (bf16
    matmuls throughout; fp8 fails the 2e-2 gate since quantization noise on
    randomly-signed sums passes straight to the output).
  - Startup barrier collective absorbs launch/clock skew before the pipeline;
    the gpsimd queue carries ONLY collectives (a collective blocks its queue
    until completion, so nothing compute-critical may sit behind one).
  - QKV projections grouped q/k/v across both heads (one [128,512] bias add
    per group on DVE).
  - Scores [k,q] with two heads packed via PE row tiling, emitted per
    ki-PAIR with per-parity [128,2,512] PSUM tiles (ring of 2) so the next
    pair's scores overlap this pair's exp. Causal masking is ADDITIVE (-60)
    on PSUM before exp; diagonal parities stream only from their own
    boundary column.
  - exp on ACT with a -2.0 bias; PV accumulates [65,512] per head with a
    ones column in vn (fp-transposed V tiles) producing the softmax
    denominator in row 64.
  - Normalize: den rows -> one K=33 PE matmul broadcast (emat) -> one DVE
    reciprocal -> per-head mult straight from PSUM -> one cc-send per qi.
  - One AllToAll per (batch, half); at-receives prefetched on sync early in
    the next batch; output projection + out stores ride the fill slots.
  - Software pipeline: every ki-pair consumes one "fill" unit (proj groups
    of the next batch, vtrans pairs, oproj units) to keep PE fed during exp
    round trips; b3 runs qi order (2,3,0,1) so the kernel ends on the small
    half; oproj work is deferred two batches so its A2A has always landed.
"""

import sys

sys.path.insert(0, "/opt/trn_rl_repo")

from collections import deque

import numpy as np

import concourse.bass as bass
import concourse.mybir as mybir
import concourse.tile as tile
from concourse import bacc, bass_utils

FP = mybir.dt.float32
BF = mybir.dt.bfloat16
F8 = mybir.dt.float8e4
AOP = mybir.AluOpType
AFT = mybir.ActivationFunctionType
DR = mybir.MatmulPerfMode.DoubleRow

B, S, D, H = 4, 2048, 1024, 16
HD = 64
N_CORES = 8
NT = B * S  # 8192 tokens
TOK_PER_CORE = NT // N_CORES  # 1024
KD = D // 128  # 8 contraction tiles for the projections
VN_BUFS = 10
EXP_BIAS = -2.0
import os
PV_MODE = os.environ.get("PV_MODE", "bf16")  # "bf16" | "fp8" | "dr"
USE_BARRIER = os.environ.get("USE_BARRIER", "1") == "1"
AT_Q = os.environ.get("AT_Q", "sync")  # "gpsimd" | "sync"
DEBUG_DUMP = os.environ.get("DEBUG_DUMP", "0") == "1"
ES_DT = BF if PV_MODE == "bf16" else F8


def build_nc():
    nc = bacc.Bacc(None, target_bir_lowering=False, debug=False, num_devices=N_CORES)

    xt = nc.dram_tensor("xt", [16, 128, KD, 512], BF, kind="ExternalInput")
    wqkv = nc.dram_tensor("wqkv", [128, 3 * KD, 128], BF, kind="ExternalInput")
    bqkv = nc.dram_tensor("bqkv", [128, 3], FP, kind="ExternalInput")
    wo = nc.dram_tensor("wo", [128, KD, D], BF, kind="ExternalInput")
    nmaskd = nc.dram_tensor("nmask", [128, 2, 896], BF, kind="ExternalInput")
    identd = nc.dram_tensor("ident", [128, 128], BF, kind="ExternalInput")
    out = nc.dram_tensor("out", [TOK_PER_CORE, D], FP, kind="ExternalOutput")
    dbg = {}
    if DEBUG_DUMP:
        for nm, shape, dt in [
            ("dbg_q", [128, S], BF), ("dbg_k", [128, S], BF), ("dbg_v", [128, S], BF),
            ("dbg_vn", [128, 2, 2, 72], ES_DT), ("dbg_es", [128, 2, 2, 512], ES_DT),
            ("dbg_dn0", [1, 512], FP), ("dbg_bc", [128, 512], FP),
            ("dbg_rc", [128, 512], FP), ("dbg_on", [128, 512], BF),
            ("dbg_at", [128, N_CORES, 128], BF),
        ]:
            dbg[nm] = nc.dram_tensor(nm, shape, dt, kind="ExternalOutput")

    with tile.TileContext(nc) as tc:
        with (
            tc.tile_pool(name="const", bufs=1) as const,
            tc.tile_pool(name="xtp", bufs=3) as xtp,
            tc.tile_pool(name="qkv", bufs=2) as qkv,
            tc.tile_pool(name="vnp", bufs=VN_BUFS) as vnp,
            tc.tile_pool(name="esp", bufs=2) as esp,
            tc.tile_pool(name="small", bufs=2) as small,
            tc.tile_pool(name="actp", bufs=4) as actp,
            tc.tile_pool(name="oop", bufs=2) as oop,
            tc.tile_pool(name="ps_s", bufs=2, space="PSUM") as ps_s,
            tc.tile_pool(name="ps_o", bufs=1, space="PSUM") as ps_o,
            tc.tile_pool(name="ps_mm", bufs=2, space="PSUM") as ps_mm,
            tc.tile_pool(name="dram", bufs=1, space="DRAM") as dram,
        ):
            cc_ins = [
                [
                    dram.tile([N_CORES, 128, 128], BF, name=f"cc_in{b}_{hf}")
                    for hf in range(2)
                ]
                for b in range(B)
            ]
            cc_outs = [
                [
                    dram.tile([N_CORES, 128, 128], BF, name=f"cc_out{b}_{hf}")
                    for hf in range(2)
                ]
                for b in range(B)
            ]
            bar_i = dram.tile([N_CORES, 128], FP, name="bar_i")
            bar_o = dram.tile([N_CORES, 128], FP, name="bar_o")

            # startup barrier: first collective absorbs launch skew + channel
            # setup while the prologue computes
            if USE_BARRIER:
                nc.gpsimd.collective_compute(
                    "AllToAll",
                    AOP.bypass,
                    replica_groups=[list(range(N_CORES))],
                    ins=[bar_i[:].opt()],
                    outs=[bar_o[:].opt()],
                )

            # PE warmup on junk data while the first DMAs land (HAM ramp)
            warm = const.tile([128, 640], BF, name="warm")
            nc.vector.memset(warm[:], 0.25)
            wps = ps_mm.tile([128, 512], FP, name="wps", tag="mm")
            for w in range(16):
                nc.tensor.matmul(
                    wps[:],
                    lhsT=warm[:, 0:128],
                    rhs=warm[:, 128:640],
                    start=(w == 0),
                    stop=(w == 15),
                )

            # ---- resident constants (scalar queue) ----
            wqkv_sb = const.tile([128, 3 * KD, 128], BF, name="wqkv_sb")
            nc.scalar.dma_start(wqkv_sb[:], wqkv[:])
            bq_sb = const.tile([128, 3], FP, name="bq_sb")
            nc.scalar.dma_start(bq_sb[:], bqkv[:])
            ebias = const.tile([128, 1], FP, name="ebias")
            nc.vector.memset(ebias[:], EXP_BIAS)
            emat = const.tile([33, 128], BF, name="emat")
            nc.vector.memset(emat[:], 0.0)
            nc.vector.memset(emat[0:1, 0:64], 1.0)
            nc.vector.memset(emat[32:33, 64:128], 1.0)
            wo_sb = const.tile([128, KD, D], BF, name="wo_sb")

            qkv_tiles = {}
            vn_tiles = {}
            xt_tiles = {}
            at_tiles = {}

            def alloc_qkv(b):
                qkv_tiles[b] = (
                    qkv.tile([128, S], BF, name="qT", tag="qT"),
                    qkv.tile([128, S], BF, name="kT", tag="kT"),
                    qkv.tile([128, S], BF, name="vT", tag="vT"),
                )

            def emit_xt_load(b, st):
                t = xtp.tile([128, KD, 512], BF, name="xt_st", tag="xt")
                nc.scalar.dma_start(t[:], xt[4 * b + st])
                xt_tiles[(b, st)] = t

            def emit_proj_group(b, st, g):
                xt_st = xt_tiles[(b, st)]
                ps = ps_mm.tile([128, 512], FP, name="ps_p", tag="mm")
                for kd in range(KD):
                    nc.tensor.matmul(
                        ps[:],
                        lhsT=wqkv_sb[:, g * KD + kd, :],
                        rhs=xt_st[:, kd, :],
                        start=(kd == 0),
                        stop=(kd == KD - 1),
                    )
                dst = qkv_tiles[b][g]
                nc.vector.tensor_scalar(
                    dst[:, st * 512 : (st + 1) * 512],
                    ps[:],
                    bq_sb[:, g : g + 1],
                    None,
                    AOP.add,
                )

            def init_vtrans(b):
                vn_tiles[b] = {}

            def emit_vtrans_pair(b, jp):
                _, _, vT = qkv_tiles[b]
                vn = vnp.tile([128, 2, 2, 72], ES_DT, name="vn", tag="vn")
                # fill with 1.0 first; m=64 stays as the denominator ones
                nc.vector.memset(vn[:, :, :, 64:65], 1.0)
                for p in range(2):
                    pst = ps_mm.tile([128, 128], BF, name="ps_t", tag="mm")
                    kc = 2 * jp + p
                    nc.tensor.transpose(
                        pst[:], vT[:, kc * 128 : (kc + 1) * 128], ident_sb[:]
                    )
                    nc.vector.tensor_copy(
                        out=vn[:, p, :, 0:64],
                        in_=pst[:].rearrange("p (h m) -> p h m", h=2),
                    )
                vn_tiles[b][jp] = vn

            def emit_a2a(bb, hf):
                nc.gpsimd.collective_compute(
                    "AllToAll",
                    AOP.bypass,
                    replica_groups=[list(range(N_CORES))],
                    ins=[cc_ins[bb][hf][:].opt()],
                    outs=[cc_outs[bb][hf][:].opt()],
                )


            def emit_at_fetch(bb, pos):
                at = actp.tile([128, N_CORES, 128], BF, name="at", tag="at")
                nc.sync.dma_start(
                    at[:], cc_outs[bb][pos][:].rearrange("f p t -> p f t")
                )
                at_tiles[(bb, pos)] = at

            def emit_oproj_unit(bb, pos, nn):
                at = at_tiles[(bb, pos)]
                if DEBUG_DUMP and bb == 0 and pos == 0 and nn == 0:
                    nc.scalar.dma_start(dbg["dbg_at"][:], at[:])
                ps = ps_mm.tile([128, 512], FP, name="ps_op", tag="mm")
                for ft in range(N_CORES):
                    nc.tensor.matmul(
                        ps[:],
                        lhsT=at[:, ft, :],
                        rhs=wo_sb[:, ft, nn * 512 : (nn + 1) * 512],
                        start=(ft == 0),
                        stop=(ft == N_CORES - 1),
                    )
                row0 = (2 * bb + pos) * 128
                oo = oop.tile([128, 512], FP, name="oo", tag="oo")
                nc.vector.tensor_copy(out=oo[:], in_=ps[:])
                nc.sync.dma_start(
                    out[row0 : row0 + 128, nn * 512 : (nn + 1) * 512], oo[:]
                )

            def emit_pv(b, qi, j, es, cps, po):
                for h in range(2):
                    for p in range(2):
                        nc.tensor.matmul(
                            po[h][:, cps[p] : 512],
                            lhsT=vn_tiles[b][j][:, p, h, 0:65],
                            rhs=es[:, p, h, cps[p] : 512],
                            start=(j == 0 and p == 0),
                            stop=(j == 2 * (qi + 1) - 1 and p == 1),
                        )

            def emit_attn_qi(b, qi, fill):
                qT, kT, _ = qkv_tiles[b]
                po = [
                    ps_o.tile([65, 512], FP, name=f"po{h}", tag=f"o{h}")
                    for h in range(2)
                ]
                NP = 2 * (qi + 1)
                prev = None
                for j in range(NP):
                    jr = j - 2 * qi
                    # per-parity first column: even starts at its boundary,
                    # odd one 128-block later
                    if jr >= 0:
                        cps = (256 * jr, 256 * jr + 128)
                    else:
                        cps = (0, 0)
                    es = esp.tile([128, 2, 2, 512], ES_DT, name="es", tag="es")
                    for p in range(2):
                        # per-parity psum tile; bufs=2 ring lets scores of the
                        # next pair overlap this pair's exp
                        sc = ps_s.tile([128, 2, 512], FP, name="sc", tag="sc")
                        ki = 2 * j + p
                        for h in range(2):
                            nc.tensor.matmul(
                                sc[:, h, cps[p] : 512],
                                lhsT=kT[h * 64 : (h + 1) * 64, ki * 128 : (ki + 1) * 128],
                                rhs=qT[
                                    h * 64 : (h + 1) * 64,
                                    qi * 512 + cps[p] : (qi + 1) * 512,
                                ],
                                start=True,
                                stop=True,
                                tile_position=(h * 64, 0),
                            )
                        if jr >= 0:
                            nc.vector.tensor_tensor(
                                sc[:, :, cps[p] : cps[p] + 128],
                                sc[:, :, cps[p] : cps[p] + 128],
                                nmask_sb[:, :, 384:512],
                                AOP.add,
                            )
                        nc.scalar.activation(
                            es[:, p, :, cps[p] : 512],
                            sc[:, :, cps[p] : 512],
                            AFT.Exp,
                            bias=ebias[:],
                        )
                    if DEBUG_DUMP and b == 0 and qi == 0 and j == 0:
                        nc.scalar.dma_start(dbg["dbg_es"][:], es[:])
                    if prev is not None:
                        emit_pv(b, qi, j - 1, prev[0], prev[1], po)
                    prev = (es, cps)
                    if fill:
                        u = fill.popleft()
                        if u is not None:
                            u()
                emit_pv(b, qi, NP - 1, prev[0], prev[1], po)

                # normalize: den -> broadcast -> reciprocal -> scale
                dnb = small.tile([33, 512], BF, name="dnb", tag="dnb")
                nc.vector.memset(dnb[:], 0.0)
                nc.vector.tensor_scalar_max(dnb[0:1, :], po[0][64:65, :], 1e-30)
                nc.vector.tensor_scalar_max(dnb[32:33, :], po[1][64:65, :], 1e-30)
                # one matmul broadcasts den0 -> rows 0-63, den1 -> rows 64-127
                bcb = ps_mm.tile([128, 512], FP, name="bcb", tag="mm")
                nc.tensor.matmul(bcb[:], lhsT=emat[:], rhs=dnb[:], start=True, stop=True)
                rc = small.tile([128, 512], FP, name="rc", tag="rc")
                nc.vector.reciprocal_approx_fast(out=rc[:], in_=bcb[:])
                on = small.tile([128, 512], BF, name="on", tag="on")
                nc.vector.tensor_tensor(on[0:64, :], po[0][0:64, :], rc[0:64, :], AOP.mult)
                nc.vector.tensor_tensor(
                    on[64:128, :], po[1][0:64, :], rc[64:128, :], AOP.mult
                )
                if DEBUG_DUMP and b == 0 and qi == 0:
                    nc.scalar.dma_start(dbg["dbg_dn0"][:], dnb[0:1, :])
                    nc.scalar.dma_start(dbg["dbg_rc"][:], rc[:])
                    nc.scalar.dma_start(dbg["dbg_on"][:], on[:])
                t0r = (4 * qi) % 8
                nc.sync.dma_start(
                    cc_ins[b][qi // 2][t0r : t0r + 4, :, :].rearrange("r p t -> p r t"),
                    on[:].rearrange("p (r t) -> p r t", r=4),
                )

            # ---- prologue: only st0 of batch 0; the rest rides b0's fill
            alloc_qkv(0)
            init_vtrans(0)
            emit_xt_load(0, 0)
            nmask_sb = const.tile([128, 2, 896], BF, name="nmask_sb")
            nc.scalar.dma_start(nmask_sb[:], nmaskd[:])
            ident_sb = const.tile([128, 128], BF, name="ident_sb")
            nc.scalar.dma_start(ident_sb[:], identd[:])
            emit_xt_load(0, 1)
            for g in range(3):
                emit_proj_group(0, 0, g)
            nc.scalar.dma_start(wo_sb[:], wo[:])
            for jp in range(2):
                emit_vtrans_pair(0, jp)
            if DEBUG_DUMP:
                nc.scalar.dma_start(dbg["dbg_q"][:], qkv_tiles[0][0][:])
                nc.scalar.dma_start(dbg["dbg_k"][:], qkv_tiles[0][1][:])
                nc.scalar.dma_start(dbg["dbg_v"][:], qkv_tiles[0][2][:])
                nc.scalar.dma_start(dbg["dbg_vn"][:], vn_tiles[0][0][:])

            # ---- software-pipelined batch loop ----
            # fill slots per batch: sum NP = 2+4+6+8 = 20, consumed one per
            # ki-pair; keep PE fed during the exp round trips
            for b in range(B):
                last = b == B - 1
                fill = deque()
                if b == 0:
                    for st in (1, 2, 3):
                        def u_load0(st=st):
                            if st < 3:
                                emit_xt_load(0, st + 1)
                            emit_proj_group(0, st, 0)
                        fill.append(u_load0)
                        for g in (1, 2):
                            fill.append(lambda st=st, g=g: emit_proj_group(0, st, g))
                        fill.append(lambda st=st: emit_vtrans_pair(0, 2 * st))
                        fill.append(lambda st=st: emit_vtrans_pair(0, 2 * st + 1))
                if not last:
                    alloc_qkv(b + 1)
                    init_vtrans(b + 1)
                    for st in range(4):
                        def u_load(st=st):
                            emit_xt_load(b + 1, st)
                            emit_proj_group(b + 1, st, 0)
                        fill.append(u_load)
                        for g in (1, 2):
                            fill.append(lambda st=st, g=g: emit_proj_group(b + 1, st, g))
                    # oproj for (b-2,1) and (b-1,0): both A2As long done
                    if b >= 2:
                        fill.appendleft(lambda: emit_at_fetch(b - 2, 1))
                        for nn in range(2):
                            fill.append(lambda nn=nn: emit_oproj_unit(b - 2, 1, nn))
                    if b >= 1:
                        fill.appendleft(lambda: emit_at_fetch(b - 1, 0))
                        for nn in range(2):
                            fill.append(lambda nn=nn: emit_oproj_unit(b - 1, 0, nn))
                else:
                    # b3 (qi order 2,3,0,1): all pending oproj, oldest first,
                    # spaced so each unit's A2A has landed before its slot
                    fill.append(lambda: emit_at_fetch(1, 1))
                    for nn in range(2):
                        fill.append(lambda nn=nn: emit_oproj_unit(1, 1, nn))
                    fill.append(lambda: emit_at_fetch(2, 1))
                    for nn in range(2):
                        fill.append(lambda nn=nn: emit_oproj_unit(2, 1, nn))
                    fill.extend([None] * 2)
                    fill.append(lambda: emit_at_fetch(2, 0))
                    for nn in range(2):
                        fill.append(lambda nn=nn: emit_oproj_unit(2, 0, nn))
                    fill.extend([None] * 6)
                    fill.append(lambda: emit_at_fetch(3, 1))
                    for nn in range(2):
                        fill.append(lambda nn=nn: emit_oproj_unit(3, 1, nn))
                qi_order = (2, 3, 0, 1) if b >= 2 else (0, 1, 2, 3)
                for qi in qi_order:
                    emit_attn_qi(b, qi, fill)
                    if qi == 1:
                        emit_a2a(b, 0)
                    if b >= 2 and qi == 3:
                        emit_a2a(b, 1)
                if b < 2:
                    emit_a2a(b, 1)
                while fill:
                    u = fill.popleft()
                    if u is not None:
                        u()
                if not last:
                    for jp in range(8):
                        emit_vtrans_pair(b + 1, jp)
            emit_at_fetch(B - 1, 0)
            for nn in range(2):
                emit_oproj_unit(B - 1, 0, nn)

    nc.finalize()
    return nc


_NC_CACHE = None


def _get_nc():
    global _NC_CACHE
    if _NC_CACHE is None:
        _NC_CACHE = build_nc()
    return _NC_CACHE


def make_in_maps(x, Wqkv, bqkv, Wo):
    import ml_dtypes

    bf16 = ml_dtypes.bfloat16
    scale = HD ** -0.5
    xT = x.reshape(NT, D).T.astype(bf16)  # [D, NT]
    xtn = np.ascontiguousarray(
        xT.reshape(KD, 128, 16, 512).transpose(2, 1, 0, 3)
    )  # [slab, p, kd, t]
    maskb = np.arange(896)[None, :] - 384 >= np.arange(128)[:, None]
    nmask = np.where(maskb, 0.0, -60.0).astype(bf16)
    nmask2 = np.ascontiguousarray(np.stack([nmask, nmask], axis=1))  # [128,2,896]
    ident = np.eye(128, dtype=np.float32).astype(bf16)
    wo = np.ascontiguousarray(Wo.astype(bf16).reshape(KD, 128, D).transpose(1, 0, 2))
    in_maps = []
    for c in range(N_CORES):
        h0, h1 = 2 * c, 2 * c + 1
        # grouped q/k/v weights across both heads: [3, D, 128]
        wq = np.concatenate([Wqkv[h0][:, 0:64] * scale, Wqkv[h1][:, 0:64] * scale], 1)
        wk = np.concatenate([Wqkv[h0][:, 64:128], Wqkv[h1][:, 64:128]], 1)
        wv = np.concatenate([Wqkv[h0][:, 128:192], Wqkv[h1][:, 128:192]], 1)
        wg = np.stack([wq, wk, wv]).astype(bf16)  # [3, D, 128]
        wg = (
            wg.reshape(3, KD, 128, 128).transpose(2, 0, 1, 3).reshape(128, 3 * KD, 128)
        )
        bq = np.concatenate([bqkv[h0][0:64] * scale, bqkv[h1][0:64] * scale])
        bk = np.concatenate([bqkv[h0][64:128], bqkv[h1][64:128]])
        bv = np.concatenate([bqkv[h0][128:192], bqkv[h1][128:192]])
        bg = np.stack([bq, bk, bv], axis=1).astype(np.float32)  # [128, 3]
        in_maps.append(
            {
                "xt": xtn,
                "wqkv": np.ascontiguousarray(wg),
                "bqkv": np.ascontiguousarray(bg),
                "wo": wo,
                "nmask": nmask2,
                "ident": ident,
            }
        )
    return in_maps


def run_cores(in_maps, trace=False, trace_kwargs=None):
    nc = _get_nc()
    kwargs = {}
    if trace:
        kwargs["trace"] = True
        if trace_kwargs:
            kwargs["trace_kwargs"] = trace_kwargs
    return bass_utils.run_bass_kernel_spmd(
        nc, in_maps, core_ids=list(range(N_CORES)), **kwargs
    )


def assemble(results, bo):
    """Reassemble core outputs (interleaved token-tile mapping) into [B,S,D]."""
    full = np.empty((NT, D), np.float32)
    for c in range(N_CORES):
        o = results[c]["out"]
        for b in range(B):
            for pos in range(2):
                t = c + 8 * pos  # token tile within batch
                dst = b * S + t * 128
                full[dst : dst + 128] = o[(2 * b + pos) * 128 : (2 * b + pos + 1) * 128]
    full += bo[None, :]
    return full.reshape(B, S, D)


def kernel(x, Wqkv, bqkv, Wo, bo):
    x = np.asarray(x, dtype=np.float32)
    Wqkv = np.asarray(Wqkv, dtype=np.float32)
    bqkv = np.asarray(bqkv, dtype=np.float32)
    Wo = np.asarray(Wo, dtype=np.float32)
    bo = np.asarray(bo, dtype=np.float32)

    in_maps = make_in_maps(x, Wqkv, bqkv, Wo)
    res = run_cores(in_maps)
    return assemble(res.results, bo)
